# revision 11
# baseline (speedup 1.0000x reference)
"""Trainium2 Bass kernel: scatter rho[b, i, j] -> out[b, fock_idx[i], fock_idx[j]].

Sharding: batch dim B across the 8 NeuronCores (pure data parallel). fock_idx is
known on the host at call time, so the scatter addressing is baked into the
compiled program as static DMA/compute access patterns.

Per-core algorithm (out is [D, D], zero except out[idx[i], idx[j]] = rho[i, j]):
  - The runtime hands the NEFF a zero-initialized ExternalOutput buffer (both
    the native run_neff path and the axon/bass2jax donation path guarantee
    this), so only rows/columns that receive data are written.
  - fock_idx (for the real problem) is strictly increasing and decomposes into
    runs of consecutive indices (32 runs of 32). Columns: each rho row is
    expanded into a [span]-wide row in SBUF with the runs placed at their
    target offsets and zeros in the gaps. Rows: each 128-row tile of rho is
    stored with one DMA per row-run to the matching block of out rows,
    touching only columns [c0, c1).
  - The W expansion buffers are memset once up front and reused cyclically:
    the gap columns stay zero across reuse because the per-tile copies only
    ever write the (fixed) data columns.
  - Expansion copies run on Vector (single runs) and GpSimd (pair-merged
    runs); stores alternate between the two HWDGE rings (SP and ACT) so
    DMA issue is not serialized on one sequencer.
"""

import numpy as np

import concourse.bacc as bacc
import concourse.bass as bass
import concourse.mybir as mybir
from concourse import tile
from concourse.bass_utils import run_bass_kernel_spmd

N_CORES = 8
P = 128  # SBUF partitions
W_BUFS = 4
R_BUFS = 4


def _runs(dst, src):
    """Maximal runs where dst and src both advance by 1. Yields (d0, s0, len)."""
    out = []
    d0, s0, L = int(dst[0]), int(src[0]), 1
    for k in range(1, len(dst)):
        if int(dst[k]) == d0 + L and int(src[k]) == s0 + L:
            L += 1
        else:
            out.append((d0, s0, L))
            d0, s0, L = int(dst[k]), int(src[k]), 1
    out.append((d0, s0, L))
    return out


def _pair_runs(col_runs):
    """Group adjacent equal-length runs into stride-2 pairs.

    Returns a list of (dst0, src0, pair_dst_stride, pair_src_stride, n, L)
    where n is 1 or 2 repeats of an L-wide copy.
    """
    out = []
    k = 0
    while k < len(col_runs):
        d0, s0, L = col_runs[k]
        if k + 1 < len(col_runs) and col_runs[k + 1][2] == L:
            d1, s1, _ = col_runs[k + 1]
            out.append((d0, s0, d1 - d0, s1 - s0, 2, L))
            k += 2
        else:
            out.append((d0, s0, L, L, 1, L))
            k += 1
    return out


def _build(idx, D, n):
    """Build the per-core Bass program with idx baked in."""
    f32 = mybir.dt.float32

    # Column placement: process columns in sorted-index order so the SBUF row
    # image is written left to right; a run needs source columns contiguous too.
    order = np.argsort(idx, kind="stable")
    col_runs = _runs(idx[order], order)  # (dst_col, src_col, len)
    c0 = min(r[0] for r in col_runs)
    c1 = max(r[0] + r[2] for r in col_runs)
    span = c1 - c0

    # ~20/32 runs to Vector as singles; 12 to Scalar as pair-merged copies.
    # GpSimd stays idle: its SBUF port is an exclusive lock shared with DVE,
    # and concurrent GpSimd copies double every DVE op's latency.
    runs_v = [r for k, r in enumerate(col_runs) if k % 8 < 5]
    pairs_s = _pair_runs([r for k, r in enumerate(col_runs) if k % 8 >= 5])

    nc = bacc.Bacc("TRN2", target_bir_lowering=False, debug=False,
                   num_devices=N_CORES)
    rho = nc.dram_tensor("rho", [n, n], f32, kind="ExternalInput")
    out = nc.dram_tensor("out", [D, D], f32, kind="ExternalOutput")

    n_tiles = (n + P - 1) // P
    with tile.TileContext(nc) as tc:
        with (
            tc.tile_pool(name="rp", bufs=R_BUFS) as rp,
            tc.tile_pool(name="wp", bufs=1) as wp,
        ):
            # W expansion buffers, memset once and reused cyclically. All on
            # Vector (GpSimd would contend for the shared SBUF port), with
            # only W0 up front and the rest staggered behind early tiles.
            ws = [wp.tile([P, span], f32, name=f"W{k}") for k in range(W_BUFS)]
            nc.vector.memset(ws[0][:], 0.0)
            next_memset = 1

            n_store = 0

            for t in range(n_tiles):
                r0 = t * P
                rows = min(P, n - r0)
                R = rp.tile([P, n], f32)
                nc.sync.dma_start(R[:rows, :], rho[r0:r0 + rows, :])

                W = ws[t % W_BUFS]
                for d0, s0, L in runs_v:
                    nc.vector.tensor_copy(W[:rows, d0 - c0:d0 - c0 + L],
                                          R[:rows, s0:s0 + L])
                for d0, s0, ds, ss, cnt, L in pairs_s:
                    dst = bass.AP(W.tensor, W.offset + (d0 - c0),
                                  [[W.ap[0][0], rows], [ds, cnt], [1, L]])
                    src = bass.AP(R.tensor, R.offset + s0,
                                  [[R.ap[0][0], rows], [ss, cnt], [1, L]])
                    nc.scalar.copy(dst, src)

                # Row runs within this tile: consecutive rho rows with
                # consecutive target rows share one store DMA. Stores split
                # 5:3 between the SP and ACT HWDGE rings (ACT also runs the
                # pair copies, SP also issues the loads).
                for dr, sr, L in _runs(idx[r0:r0 + rows], range(rows)):
                    ring = nc.sync if n_store % 8 < 5 else nc.scalar
                    n_store += 1
                    ring.dma_start(out[dr:dr + L, c0:c1], W[sr:sr + L, :])

                # Stagger the remaining one-time memsets behind early tiles.
                while next_memset < W_BUFS and next_memset <= t + 1:
                    nc.vector.memset(ws[next_memset][:], 0.0)
                    next_memset += 1
    nc.compile()
    return nc


def kernel(input_state, fock_idx, fock_dim):
    input_state = np.asarray(input_state)
    idx = np.asarray(fock_idx).astype(np.int64)
    D = int(fock_dim)
    B, n, _ = input_state.shape

    nc = _build(idx, D, n)

    out = np.empty((B, D, D), dtype=input_state.dtype)
    for start in range(0, B, N_CORES):
        stop = min(start + N_CORES, B)
        in_maps = [
            {"rho": np.ascontiguousarray(input_state[b], dtype=np.float32)}
            for b in range(start, stop)
        ]
        res = run_bass_kernel_spmd(nc, in_maps,
                                   core_ids=list(range(stop - start)))
        for k, b in enumerate(range(start, stop)):
            out[b] = res.results[k]["out"]
    return out


# revision 17
# speedup vs baseline: 1.0135x; 1.0135x over previous
"""Trainium2 Bass kernel: scatter rho[b, i, j] -> out[b, fock_idx[i], fock_idx[j]].

Sharding: batch dim B across the 8 NeuronCores (pure data parallel). fock_idx is
known on the host at call time, so the scatter addressing is baked into the
compiled program as static DMA/compute access patterns.

Per-core algorithm (out is [D, D], zero except out[idx[i], idx[j]] = rho[i, j]):
  - The runtime hands the NEFF a zero-initialized ExternalOutput buffer (both
    the native run_neff path and the axon/bass2jax donation path guarantee
    this), so only rows/columns that receive data are written.
  - fock_idx (for the real problem) is strictly increasing and decomposes into
    runs of consecutive indices (32 runs of 32). Columns: each rho row is
    expanded into a [span]-wide row in SBUF with the runs placed at their
    target offsets and zeros in the gaps. Rows: each 128-row tile of rho is
    stored with one DMA per row-run to the matching block of out rows,
    touching only columns [c0, c1).
  - The W expansion buffers are memset once up front and reused cyclically:
    the gap columns stay zero across reuse because the per-tile copies only
    ever write the (fixed) data columns.
  - Expansion copies run on Vector (single runs) and GpSimd (pair-merged
    runs); stores alternate between the two HWDGE rings (SP and ACT) so
    DMA issue is not serialized on one sequencer.
"""

import numpy as np

import concourse.bacc as bacc
import concourse.bass as bass
import concourse.mybir as mybir
from concourse import tile
from concourse.bass_utils import run_bass_kernel_spmd

N_CORES = 8
P = 128  # SBUF partitions
W_BUFS = 4
R_BUFS = 4


def _runs(dst, src):
    """Maximal runs where dst and src both advance by 1. Yields (d0, s0, len)."""
    out = []
    d0, s0, L = int(dst[0]), int(src[0]), 1
    for k in range(1, len(dst)):
        if int(dst[k]) == d0 + L and int(src[k]) == s0 + L:
            L += 1
        else:
            out.append((d0, s0, L))
            d0, s0, L = int(dst[k]), int(src[k]), 1
    out.append((d0, s0, L))
    return out


def _pair_runs(col_runs):
    """Group adjacent equal-length runs into stride-2 pairs.

    Returns a list of (dst0, src0, pair_dst_stride, pair_src_stride, n, L)
    where n is 1 or 2 repeats of an L-wide copy.
    """
    out = []
    k = 0
    while k < len(col_runs):
        d0, s0, L = col_runs[k]
        if k + 1 < len(col_runs) and col_runs[k + 1][2] == L:
            d1, s1, _ = col_runs[k + 1]
            out.append((d0, s0, d1 - d0, s1 - s0, 2, L))
            k += 2
        else:
            out.append((d0, s0, L, L, 1, L))
            k += 1
    return out


def _build(idx, D, n):
    """Build the per-core Bass program with idx baked in."""
    f32 = mybir.dt.float32

    # Column placement: process columns in sorted-index order so the SBUF row
    # image is written left to right; a run needs source columns contiguous too.
    order = np.argsort(idx, kind="stable")
    col_runs = _runs(idx[order], order)  # (dst_col, src_col, len)
    c0 = min(r[0] for r in col_runs)
    c1 = max(r[0] + r[2] for r in col_runs)
    span = c1 - c0

    # ~20/32 runs to Vector as singles; 12 to Scalar as pair-merged copies.
    # GpSimd stays idle: its SBUF port is an exclusive lock shared with DVE,
    # and concurrent GpSimd copies double every DVE op's latency.
    runs_v = [r for k, r in enumerate(col_runs) if k % 8 < 5]
    pairs_s = _pair_runs([r for k, r in enumerate(col_runs) if k % 8 >= 5])

    nc = bacc.Bacc("TRN2", target_bir_lowering=False, debug=False,
                   num_devices=N_CORES)
    rho = nc.dram_tensor("rho", [n, n], f32, kind="ExternalInput")
    out = nc.dram_tensor("out", [D, D], f32, kind="ExternalOutput")

    n_tiles = (n + P - 1) // P
    with tile.TileContext(nc) as tc:
        with (
            tc.tile_pool(name="rp", bufs=R_BUFS) as rp,
            tc.tile_pool(name="wp", bufs=1) as wp,
        ):
            # W expansion buffers, memset once and reused cyclically. All on
            # Vector (GpSimd would contend for the shared SBUF port), with
            # only W0 up front and the rest staggered behind early tiles.
            ws = [wp.tile([P, span], f32, name=f"W{k}") for k in range(W_BUFS)]
            nc.vector.memset(ws[0][:], 0.0)
            next_memset = 1

            n_store = 0

            for t in range(n_tiles):
                r0 = t * P
                rows = min(P, n - r0)
                R = rp.tile([P, n], f32)
                nc.sync.dma_start(R[:rows, :], rho[r0:r0 + rows, :])

                W = ws[t % W_BUFS]
                for d0, s0, L in runs_v:
                    nc.vector.tensor_copy(W[:rows, d0 - c0:d0 - c0 + L],
                                          R[:rows, s0:s0 + L])
                for d0, s0, ds, ss, cnt, L in pairs_s:
                    dst = bass.AP(W.tensor, W.offset + (d0 - c0),
                                  [[W.ap[0][0], rows], [ds, cnt], [1, L]])
                    src = bass.AP(R.tensor, R.offset + s0,
                                  [[R.ap[0][0], rows], [ss, cnt], [1, L]])
                    nc.scalar.copy(dst, src)

                # Row runs within this tile: consecutive rho rows with
                # consecutive target rows share one store DMA. Stores split
                # 5:3 between the SP and ACT HWDGE rings (ACT also runs the
                # pair copies, SP also issues the loads).
                for dr, sr, L in _runs(idx[r0:r0 + rows], range(rows)):
                    ring = nc.sync if n_store % 8 < 5 else nc.scalar
                    n_store += 1
                    ring.dma_start(out[dr:dr + L, c0:c1], W[sr:sr + L, :])

                # Stagger the remaining one-time memsets behind early tiles.
                while next_memset < W_BUFS and next_memset <= t + 1:
                    nc.vector.memset(ws[next_memset][:], 0.0)
                    next_memset += 1
    nc.compile()
    return nc


def _build_raw(idx, D, n):
    """Raw-bacc build: same pipeline as _build but with hand-rolled
    semaphores, skipping Tile's ~7us startup butterfly and ~2.5us tail."""
    f32 = mybir.dt.float32

    order = np.argsort(idx, kind="stable")
    col_runs = _runs(idx[order], order)
    c0 = min(r[0] for r in col_runs)
    c1 = max(r[0] + r[2] for r in col_runs)
    span = c1 - c0

    # ~20/32 runs to Vector as singles; 12 to Scalar as pair-merged copies.
    runs_v = [r for k, r in enumerate(col_runs) if k % 8 < 5]
    pairs_s = _pair_runs([r for k, r in enumerate(col_runs) if k % 8 >= 5])

    nc = bacc.Bacc("TRN2", target_bir_lowering=False, debug=False,
                   num_devices=N_CORES)
    rho = nc.dram_tensor("rho", [n, n], f32, kind="ExternalInput")
    out = nc.dram_tensor("out", [D, D], f32, kind="ExternalOutput")

    n_tiles = (n + P - 1) // P
    assert n % P == 0, "raw path assumes full 128-row tiles"
    NB = 4  # R and W buffer count

    # Row-run store split per tile: first half on SP, second half on ACT.
    tile_row_runs = []
    for t in range(n_tiles):
        r0 = t * P
        tile_row_runs.append(_runs(idx[r0:r0 + P], range(P)))
    stores_per_tile = len(tile_row_runs[0])
    assert all(len(x) == stores_per_tile for x in tile_row_runs)
    sp_share = stores_per_tile // 2
    total_stores = n_tiles * stores_per_tile

    import contextlib
    with contextlib.ExitStack() as ctx:
        Rs = [ctx.enter_context(nc.sbuf_tensor(f"R{k}", [P, n], f32))
              for k in range(NB)]
        Ws = [ctx.enter_context(nc.sbuf_tensor(f"W{k}", [P, span], f32))
              for k in range(NB)]
        # Per-buffer DMA sems: a single shared sem is unsound because the 16
        # per-engine +1s of concurrent DMAs interleave, so a threshold can
        # be crossed before any single DMA fully completed.
        s_lds = [ctx.enter_context(nc.semaphore(f"s_ld{k}"))
                 for k in range(NB)]                      # +16 per load of Rk
        s_sts = [ctx.enter_context(nc.semaphore(f"s_st{k}"))
                 for k in range(NB)]                      # +16 per store of Wk
        s_v = ctx.enter_context(nc.semaphore("s_v"))      # +1 per tile (V copies)
        s_a = ctx.enter_context(nc.semaphore("s_a"))      # +1 per tile (ACT copies)
        s_ms = ctx.enter_context(nc.semaphore("s_ms"))    # +1 per W memset
        block = ctx.enter_context(nc.Block())

        spt = stores_per_tile

        def w_ap(t, d0, ds, cnt, L, rows=P):
            W = Ws[t % NB]
            return bass.AP(W, (d0 - c0), [[span, rows], [ds, cnt], [1, L]])

        def r_ap(t, s0, ss, cnt, L, rows=P):
            R = Rs[t % NB]
            return bass.AP(R, s0, [[n, rows], [ss, cnt], [1, L]])

        @block.sync
        def _(sp):
            for t in range(min(NB, n_tiles)):
                sp.dma_start(Rs[t][:, :], rho[t * P:(t + 1) * P, :]
                             ).then_inc(s_lds[t], 16)
            for t in range(n_tiles):
                k, c = t % NB, t // NB
                sp.wait_ge(s_v, t + 1)
                sp.wait_ge(s_a, t + 1)
                if t < NB:
                    sp.wait_ge(s_ms, t + 1)
                else:
                    sp.wait_ge(s_sts[k], 16 * spt * c)
                W = Ws[k]
                for dr, sr, L in tile_row_runs[t][:sp_share]:
                    sp.dma_start(out[dr:dr + L, c0:c1], W[sr:sr + L, :]
                                 ).then_inc(s_sts[k], 16)
                if t + NB < n_tiles:
                    # R[k] readers (tile t's copies) just finished.
                    sp.wait_ge(s_lds[k], 16 * (c + 1))
                    sp.dma_start(Rs[k][:, :],
                                 rho[(t + NB) * P:(t + NB + 1) * P, :]
                                 ).then_inc(s_lds[k], 16)
            for k in range(NB):
                n_k = len([t for t in range(n_tiles) if t % NB == k])
                sp.wait_ge(s_sts[k], 16 * spt * n_k)

        @block.scalar
        def _(act):
            for t in range(n_tiles):
                k, c = t % NB, t // NB
                act.wait_ge(s_lds[k], 16 * (c + 1))
                if t < NB:
                    act.wait_ge(s_ms, k + 1)
                else:
                    act.wait_ge(s_sts[k], 16 * spt * c)
                    act.wait_ge(s_a, t - NB + 1)
                last = len(pairs_s) - 1
                for i, (d0, s0, ds, ss, cnt, L) in enumerate(pairs_s):
                    ins = act.copy(w_ap(t, d0, ds, cnt, L),
                                   r_ap(t, s0, ss, cnt, L))
                    if i == last:
                        ins.then_inc(s_a, 1)
                act.wait_ge(s_v, t + 1)
                act.wait_ge(s_a, t + 1)
                W = Ws[k]
                for dr, sr, L in tile_row_runs[t][sp_share:]:
                    act.dma_start(out[dr:dr + L, c0:c1], W[sr:sr + L, :]
                                  ).then_inc(s_sts[k], 16)

        @block.vector
        def _(v):
            v.memset(Ws[0][:, :], 0.0).then_inc(s_ms, 1)
            for t in range(n_tiles):
                k, c = t % NB, t // NB
                v.wait_ge(s_lds[k], 16 * (c + 1))
                if t < NB:
                    v.wait_ge(s_ms, k + 1)
                else:
                    v.wait_ge(s_sts[k], 16 * spt * c)
                    v.wait_ge(s_v, t - NB + 1)
                R, W = Rs[k], Ws[k]
                last = len(runs_v) - 1
                for i, (d0, s0, L) in enumerate(runs_v):
                    ins = v.tensor_copy(W[:, d0 - c0:d0 - c0 + L],
                                        R[:, s0:s0 + L])
                    if i == last:
                        ins.then_inc(s_v, 1)
                if t == 0:
                    v.memset(Ws[1][:, :], 0.0).then_inc(s_ms, 1)
                    v.memset(Ws[2][:, :], 0.0).then_inc(s_ms, 1)
                elif t == 1 and NB > 3:
                    v.memset(Ws[3][:, :], 0.0).then_inc(s_ms, 1)

    nc.compile()
    return nc


def kernel(input_state, fock_idx, fock_dim):
    input_state = np.asarray(input_state)
    idx = np.asarray(fock_idx).astype(np.int64)
    D = int(fock_dim)
    B, n, _ = input_state.shape

    try:
        nc = _build_raw(idx, D, n)
    except Exception:
        nc = _build(idx, D, n)

    out = np.empty((B, D, D), dtype=input_state.dtype)
    for start in range(0, B, N_CORES):
        stop = min(start + N_CORES, B)
        in_maps = [
            {"rho": np.ascontiguousarray(input_state[b], dtype=np.float32)}
            for b in range(start, stop)
        ]
        res = run_bass_kernel_spmd(nc, in_maps,
                                   core_ids=list(range(stop - start)))
        for k, b in enumerate(range(start, stop)):
            out[b] = res.results[k]["out"]
    return out


# revision 22
# speedup vs baseline: 1.0221x; 1.0085x over previous
"""Trainium2 Bass kernel: scatter rho[b, i, j] -> out[b, fock_idx[i], fock_idx[j]].

Sharding: batch dim B across the 8 NeuronCores (pure data parallel). fock_idx is
known on the host at call time, so the scatter addressing is baked into the
compiled program as static DMA/compute access patterns.

Per-core algorithm (out is [D, D], zero except out[idx[i], idx[j]] = rho[i, j]):
  - The runtime hands the NEFF a zero-initialized ExternalOutput buffer (both
    the native run_neff path and the axon/bass2jax donation path guarantee
    this), so only rows/columns that receive data are written.
  - fock_idx (for the real problem) is strictly increasing and decomposes into
    runs of consecutive indices (32 runs of 32). Columns: each rho row is
    expanded into a [span]-wide row in SBUF with the runs placed at their
    target offsets and zeros in the gaps. Rows: each 128-row tile of rho is
    stored with one DMA per row-run to the matching block of out rows,
    touching only columns [c0, c1).
  - The W expansion buffers are memset once up front and reused cyclically:
    the gap columns stay zero across reuse because the per-tile copies only
    ever write the (fixed) data columns.
  - Expansion copies run on Vector (single runs) and GpSimd (pair-merged
    runs); stores alternate between the two HWDGE rings (SP and ACT) so
    DMA issue is not serialized on one sequencer.
"""

import numpy as np

import concourse.bacc as bacc
import concourse.bass as bass
import concourse.mybir as mybir
from concourse import tile
from concourse.bass_utils import run_bass_kernel_spmd

N_CORES = 8
P = 128  # SBUF partitions
W_BUFS = 6
R_BUFS = 3  # R tiles hold two rho tiles each


def _runs(dst, src):
    """Maximal runs where dst and src both advance by 1. Yields (d0, s0, len)."""
    out = []
    d0, s0, L = int(dst[0]), int(src[0]), 1
    for k in range(1, len(dst)):
        if int(dst[k]) == d0 + L and int(src[k]) == s0 + L:
            L += 1
        else:
            out.append((d0, s0, L))
            d0, s0, L = int(dst[k]), int(src[k]), 1
    out.append((d0, s0, L))
    return out


def _pair_runs(col_runs):
    """Group adjacent equal-length runs into stride-2 pairs.

    Returns a list of (dst0, src0, pair_dst_stride, pair_src_stride, n, L)
    where n is 1 or 2 repeats of an L-wide copy.
    """
    out = []
    k = 0
    while k < len(col_runs):
        d0, s0, L = col_runs[k]
        if k + 1 < len(col_runs) and col_runs[k + 1][2] == L:
            d1, s1, _ = col_runs[k + 1]
            out.append((d0, s0, d1 - d0, s1 - s0, 2, L))
            k += 2
        else:
            out.append((d0, s0, L, L, 1, L))
            k += 1
    return out


def _build(idx, D, n):
    """Build the per-core Bass program with idx baked in."""
    f32 = mybir.dt.float32

    # Column placement: process columns in sorted-index order so the SBUF row
    # image is written left to right; a run needs source columns contiguous too.
    order = np.argsort(idx, kind="stable")
    col_runs = _runs(idx[order], order)  # (dst_col, src_col, len)
    c0 = min(r[0] for r in col_runs)
    c1 = max(r[0] + r[2] for r in col_runs)
    span = c1 - c0

    # ~18/32 runs to Vector as singles; 14 to GpSimd as pair-merged copies.
    # (Scalar is kept free to issue half the store DMAs.)
    runs_v = [r for k, r in enumerate(col_runs) if k % 16 < 9]
    pairs_g = _pair_runs([r for k, r in enumerate(col_runs) if k % 16 >= 9])

    nc = bacc.Bacc("TRN2", target_bir_lowering=False, debug=False,
                   num_devices=N_CORES)
    rho = nc.dram_tensor("rho", [n, n], f32, kind="ExternalInput")
    out = nc.dram_tensor("out", [D, D], f32, kind="ExternalOutput")

    n_tiles = (n + P - 1) // P
    with tile.TileContext(nc) as tc:
        with (
            tc.tile_pool(name="rp", bufs=R_BUFS) as rp,
            tc.tile_pool(name="wp", bufs=1) as wp,
        ):
            # W expansion buffers, memset once and reused cyclically, with
            # only W0/W1 up front and the rest staggered behind early tiles.
            ws = [wp.tile([P, span], f32, name=f"W{k}") for k in range(W_BUFS)]
            memset_eng = [nc.vector if k % 2 == 0 else nc.gpsimd
                          for k in range(W_BUFS)]
            memset_eng[0].memset(ws[0][:], 0.0)
            memset_eng[1].memset(ws[1][:], 0.0)
            next_memset = 2

            n_store = 0

            for t in range(n_tiles):
                r0 = t * P
                rows = min(P, n - r0)
                R = rp.tile([P, n], f32, name="R")
                nc.sync.dma_start(R[:rows, :], rho[r0:r0 + rows, :])

                W = ws[t % W_BUFS]
                for d0, s0, L in runs_v:
                    nc.vector.tensor_copy(
                        W[:rows, d0 - c0:d0 - c0 + L],
                        R[:rows, s0:s0 + L])
                for d0, s0, ds, ss, cnt, L in pairs_g:
                    dst = bass.AP(W.tensor, W.offset + (d0 - c0),
                                  [[W.ap[0][0], rows], [ds, cnt], [1, L]])
                    src = bass.AP(R.tensor, R.offset + s0,
                                  [[R.ap[0][0], rows], [ss, cnt], [1, L]])
                    nc.gpsimd.tensor_copy(dst, src)

                # Row runs within this tile: consecutive rho rows with
                # consecutive target rows share one store DMA, alternating
                # between the SP and ACT HWDGE rings.
                for dr, sr, L in _runs(idx[r0:r0 + rows], range(rows)):
                    ring = nc.sync if n_store % 2 == 0 else nc.scalar
                    n_store += 1
                    ring.dma_start(out[dr:dr + L, c0:c1], W[sr:sr + L, :],
                                   max_dma_last_dim=1008)

                # Stagger the remaining one-time memsets behind early tiles.
                while next_memset < W_BUFS and next_memset <= t + 2:
                    memset_eng[next_memset].memset(ws[next_memset][:], 0.0)
                    next_memset += 1
    nc.compile()
    return nc


def _build_raw(idx, D, n):
    """Raw-bacc build: same pipeline as _build but with hand-rolled
    semaphores, skipping Tile's ~7us startup butterfly and ~2.5us tail."""
    f32 = mybir.dt.float32

    order = np.argsort(idx, kind="stable")
    col_runs = _runs(idx[order], order)
    c0 = min(r[0] for r in col_runs)
    c1 = max(r[0] + r[2] for r in col_runs)
    span = c1 - c0

    # ~20/32 runs to Vector as singles; 12 to Scalar as pair-merged copies.
    runs_v = [r for k, r in enumerate(col_runs) if k % 8 < 5]
    pairs_s = _pair_runs([r for k, r in enumerate(col_runs) if k % 8 >= 5])

    nc = bacc.Bacc("TRN2", target_bir_lowering=False, debug=False,
                   num_devices=N_CORES)
    rho = nc.dram_tensor("rho", [n, n], f32, kind="ExternalInput")
    out = nc.dram_tensor("out", [D, D], f32, kind="ExternalOutput")

    n_tiles = (n + P - 1) // P
    assert n % P == 0, "raw path assumes full 128-row tiles"
    NB = 4  # R and W buffer count

    # Row-run store split per tile: first half on SP, second half on ACT.
    tile_row_runs = []
    for t in range(n_tiles):
        r0 = t * P
        tile_row_runs.append(_runs(idx[r0:r0 + P], range(P)))
    stores_per_tile = len(tile_row_runs[0])
    assert all(len(x) == stores_per_tile for x in tile_row_runs)
    sp_share = stores_per_tile // 2
    total_stores = n_tiles * stores_per_tile

    import contextlib
    with contextlib.ExitStack() as ctx:
        Rs = [ctx.enter_context(nc.sbuf_tensor(f"R{k}", [P, n], f32))
              for k in range(NB)]
        Ws = [ctx.enter_context(nc.sbuf_tensor(f"W{k}", [P, span], f32))
              for k in range(NB)]
        # Per-buffer DMA sems: a single shared sem is unsound because the 16
        # per-engine +1s of concurrent DMAs interleave, so a threshold can
        # be crossed before any single DMA fully completed.
        s_lds = [ctx.enter_context(nc.semaphore(f"s_ld{k}"))
                 for k in range(NB)]                      # +16 per load of Rk
        s_sts = [ctx.enter_context(nc.semaphore(f"s_st{k}"))
                 for k in range(NB)]                      # +16 per store of Wk
        s_v = ctx.enter_context(nc.semaphore("s_v"))      # +1 per tile (V copies)
        s_a = ctx.enter_context(nc.semaphore("s_a"))      # +1 per tile (ACT copies)
        s_ms = ctx.enter_context(nc.semaphore("s_ms"))    # +1 per W memset
        block = ctx.enter_context(nc.Block())

        spt = stores_per_tile

        def w_ap(t, d0, ds, cnt, L, rows=P):
            W = Ws[t % NB]
            return bass.AP(W, (d0 - c0), [[span, rows], [ds, cnt], [1, L]])

        def r_ap(t, s0, ss, cnt, L, rows=P):
            R = Rs[t % NB]
            return bass.AP(R, s0, [[n, rows], [ss, cnt], [1, L]])

        @block.sync
        def _(sp):
            for t in range(min(NB, n_tiles)):
                sp.dma_start(Rs[t][:, :], rho[t * P:(t + 1) * P, :]
                             ).then_inc(s_lds[t], 16)
            for t in range(n_tiles):
                k, c = t % NB, t // NB
                sp.wait_ge(s_v, t + 1)
                sp.wait_ge(s_a, t + 1)
                if t < NB:
                    sp.wait_ge(s_ms, t + 1)
                else:
                    sp.wait_ge(s_sts[k], 16 * spt * c)
                W = Ws[k]
                for dr, sr, L in tile_row_runs[t][:sp_share]:
                    sp.dma_start(out[dr:dr + L, c0:c1], W[sr:sr + L, :]
                                 ).then_inc(s_sts[k], 16)
                if t + NB < n_tiles:
                    # R[k] readers (tile t's copies) just finished.
                    sp.wait_ge(s_lds[k], 16 * (c + 1))
                    sp.dma_start(Rs[k][:, :],
                                 rho[(t + NB) * P:(t + NB + 1) * P, :]
                                 ).then_inc(s_lds[k], 16)
            for k in range(NB):
                n_k = len([t for t in range(n_tiles) if t % NB == k])
                sp.wait_ge(s_sts[k], 16 * spt * n_k)

        @block.scalar
        def _(act):
            for t in range(n_tiles):
                k, c = t % NB, t // NB
                act.wait_ge(s_lds[k], 16 * (c + 1))
                if t < NB:
                    act.wait_ge(s_ms, k + 1)
                else:
                    act.wait_ge(s_sts[k], 16 * spt * c)
                    act.wait_ge(s_a, t - NB + 1)
                last = len(pairs_s) - 1
                for i, (d0, s0, ds, ss, cnt, L) in enumerate(pairs_s):
                    ins = act.copy(w_ap(t, d0, ds, cnt, L),
                                   r_ap(t, s0, ss, cnt, L))
                    if i == last:
                        ins.then_inc(s_a, 1)
                act.wait_ge(s_v, t + 1)
                act.wait_ge(s_a, t + 1)
                W = Ws[k]
                for dr, sr, L in tile_row_runs[t][sp_share:]:
                    act.dma_start(out[dr:dr + L, c0:c1], W[sr:sr + L, :]
                                  ).then_inc(s_sts[k], 16)

        @block.vector
        def _(v):
            v.memset(Ws[0][:, :], 0.0).then_inc(s_ms, 1)
            for t in range(n_tiles):
                k, c = t % NB, t // NB
                v.wait_ge(s_lds[k], 16 * (c + 1))
                if t < NB:
                    v.wait_ge(s_ms, k + 1)
                else:
                    v.wait_ge(s_sts[k], 16 * spt * c)
                    v.wait_ge(s_v, t - NB + 1)
                R, W = Rs[k], Ws[k]
                last = len(runs_v) - 1
                for i, (d0, s0, L) in enumerate(runs_v):
                    ins = v.tensor_copy(W[:, d0 - c0:d0 - c0 + L],
                                        R[:, s0:s0 + L])
                    if i == last:
                        ins.then_inc(s_v, 1)
                if t == 0:
                    v.memset(Ws[1][:, :], 0.0).then_inc(s_ms, 1)
                    v.memset(Ws[2][:, :], 0.0).then_inc(s_ms, 1)
                elif t == 1 and NB > 3:
                    v.memset(Ws[3][:, :], 0.0).then_inc(s_ms, 1)

    nc.compile()
    return nc


def kernel(input_state, fock_idx, fock_dim):
    input_state = np.asarray(input_state)
    idx = np.asarray(fock_idx).astype(np.int64)
    D = int(fock_dim)
    B, n, _ = input_state.shape

    nc = _build(idx, D, n)

    out = np.empty((B, D, D), dtype=input_state.dtype)
    for start in range(0, B, N_CORES):
        stop = min(start + N_CORES, B)
        in_maps = [
            {"rho": np.ascontiguousarray(input_state[b], dtype=np.float32)}
            for b in range(start, stop)
        ]
        res = run_bass_kernel_spmd(nc, in_maps,
                                   core_ids=list(range(stop - start)))
        for k, b in enumerate(range(start, stop)):
            out[b] = res.results[k]["out"]
    return out


# revision 24
# speedup vs baseline: 1.0361x; 1.0137x over previous
"""Trainium2 Bass kernel: scatter rho[b, i, j] -> out[b, fock_idx[i], fock_idx[j]].

Sharding: batch dim B across the 8 NeuronCores (pure data parallel). fock_idx is
known on the host at call time, so the scatter addressing is baked into the
compiled program as static DMA/compute access patterns.

Per-core algorithm (out is [D, D], zero except out[idx[i], idx[j]] = rho[i, j]):
  - The runtime hands the NEFF a zero-initialized ExternalOutput buffer (both
    the native run_neff path and the axon/bass2jax donation path guarantee
    this), so only rows/columns that receive data are written.
  - fock_idx (for the real problem) is strictly increasing and decomposes into
    runs of consecutive indices (32 runs of 32). Columns: each rho row is
    expanded into a [span]-wide row in SBUF with the runs placed at their
    target offsets and zeros in the gaps. Rows: each 128-row tile of rho is
    stored with one DMA per row-run to the matching block of out rows,
    touching only columns [c0, c1).
  - The W expansion buffers are memset once up front and reused cyclically:
    the gap columns stay zero across reuse because the per-tile copies only
    ever write the (fixed) data columns.
  - Expansion copies run on Vector (single runs) and GpSimd (pair-merged
    runs); stores alternate between the two HWDGE rings (SP and ACT) so
    DMA issue is not serialized on one sequencer.
"""

import numpy as np

import concourse.bacc as bacc
import concourse.bass as bass
import concourse.mybir as mybir
from concourse import tile
from concourse.bass_utils import run_bass_kernel_spmd

N_CORES = 8
P = 128  # SBUF partitions
W_BUFS = 8
R_BUFS = 3  # R tiles hold two rho tiles each


def _runs(dst, src):
    """Maximal runs where dst and src both advance by 1. Yields (d0, s0, len)."""
    out = []
    d0, s0, L = int(dst[0]), int(src[0]), 1
    for k in range(1, len(dst)):
        if int(dst[k]) == d0 + L and int(src[k]) == s0 + L:
            L += 1
        else:
            out.append((d0, s0, L))
            d0, s0, L = int(dst[k]), int(src[k]), 1
    out.append((d0, s0, L))
    return out


def _pair_runs(col_runs):
    """Group adjacent equal-length runs into stride-2 pairs.

    Returns a list of (dst0, src0, pair_dst_stride, pair_src_stride, n, L)
    where n is 1 or 2 repeats of an L-wide copy.
    """
    out = []
    k = 0
    while k < len(col_runs):
        d0, s0, L = col_runs[k]
        if k + 1 < len(col_runs) and col_runs[k + 1][2] == L:
            d1, s1, _ = col_runs[k + 1]
            out.append((d0, s0, d1 - d0, s1 - s0, 2, L))
            k += 2
        else:
            out.append((d0, s0, L, L, 1, L))
            k += 1
    return out


def _build(idx, D, n):
    """Build the per-core Bass program with idx baked in."""
    f32 = mybir.dt.float32

    # Column placement: process columns in sorted-index order so the SBUF row
    # image is written left to right; a run needs source columns contiguous too.
    order = np.argsort(idx, kind="stable")
    col_runs = _runs(idx[order], order)  # (dst_col, src_col, len)
    c0 = min(r[0] for r in col_runs)
    c1 = max(r[0] + r[2] for r in col_runs)
    span = c1 - c0

    # ~18/32 runs to Vector as singles; 14 to GpSimd as pair-merged copies.
    # (Scalar is kept free to issue half the store DMAs.)
    runs_v = [r for k, r in enumerate(col_runs) if k % 16 < 9]
    pairs_g = _pair_runs([r for k, r in enumerate(col_runs) if k % 16 >= 9])

    nc = bacc.Bacc("TRN2", target_bir_lowering=False, debug=False,
                   num_devices=N_CORES)
    rho = nc.dram_tensor("rho", [n, n], f32, kind="ExternalInput")
    out = nc.dram_tensor("out", [D, D], f32, kind="ExternalOutput")

    n_tiles = (n + P - 1) // P
    with tile.TileContext(nc) as tc:
        with (
            tc.tile_pool(name="rp", bufs=R_BUFS) as rp,
            tc.tile_pool(name="wp", bufs=1) as wp,
        ):
            # W expansion buffers, memset once and reused cyclically, with
            # only W0/W1 up front and the rest staggered behind early tiles.
            ws = [wp.tile([P, span], f32, name=f"W{k}") for k in range(W_BUFS)]
            memset_eng = [nc.vector if k % 2 == 0 else nc.gpsimd
                          for k in range(W_BUFS)]
            memset_eng[0].memset(ws[0][:], 0.0)
            memset_eng[1].memset(ws[1][:], 0.0)
            next_memset = 2

            n_store = 0

            for t in range(n_tiles):
                r0 = t * P
                rows = min(P, n - r0)
                R = rp.tile([P, n], f32, name="R")
                nc.sync.dma_start(R[:rows, :], rho[r0:r0 + rows, :])

                W = ws[t % W_BUFS]
                for d0, s0, L in runs_v:
                    nc.vector.tensor_copy(
                        W[:rows, d0 - c0:d0 - c0 + L],
                        R[:rows, s0:s0 + L])
                for d0, s0, ds, ss, cnt, L in pairs_g:
                    dst = bass.AP(W.tensor, W.offset + (d0 - c0),
                                  [[W.ap[0][0], rows], [ds, cnt], [1, L]])
                    src = bass.AP(R.tensor, R.offset + s0,
                                  [[R.ap[0][0], rows], [ss, cnt], [1, L]])
                    nc.gpsimd.tensor_copy(dst, src)

                # Row runs within this tile: consecutive rho rows with
                # consecutive target rows share one store DMA, alternating
                # between the SP and ACT HWDGE rings.
                for dr, sr, L in _runs(idx[r0:r0 + rows], range(rows)):
                    ring = nc.sync if n_store % 2 == 0 else nc.scalar
                    n_store += 1
                    ring.dma_start(out[dr:dr + L, c0:c1], W[sr:sr + L, :])

                # Stagger the remaining one-time memsets behind early tiles.
                while next_memset < W_BUFS and next_memset <= t + 2:
                    memset_eng[next_memset].memset(ws[next_memset][:], 0.0)
                    next_memset += 1
    nc.compile()
    return nc


def _build_raw(idx, D, n):
    """Raw-bacc build: same pipeline as _build but with hand-rolled
    semaphores, skipping Tile's ~7us startup butterfly and ~2.5us tail."""
    f32 = mybir.dt.float32

    order = np.argsort(idx, kind="stable")
    col_runs = _runs(idx[order], order)
    c0 = min(r[0] for r in col_runs)
    c1 = max(r[0] + r[2] for r in col_runs)
    span = c1 - c0

    # ~20/32 runs to Vector as singles; 12 to Scalar as pair-merged copies.
    runs_v = [r for k, r in enumerate(col_runs) if k % 8 < 5]
    pairs_s = _pair_runs([r for k, r in enumerate(col_runs) if k % 8 >= 5])

    nc = bacc.Bacc("TRN2", target_bir_lowering=False, debug=False,
                   num_devices=N_CORES)
    rho = nc.dram_tensor("rho", [n, n], f32, kind="ExternalInput")
    out = nc.dram_tensor("out", [D, D], f32, kind="ExternalOutput")

    n_tiles = (n + P - 1) // P
    assert n % P == 0, "raw path assumes full 128-row tiles"
    NB = 4  # R and W buffer count

    # Row-run store split per tile: first half on SP, second half on ACT.
    tile_row_runs = []
    for t in range(n_tiles):
        r0 = t * P
        tile_row_runs.append(_runs(idx[r0:r0 + P], range(P)))
    stores_per_tile = len(tile_row_runs[0])
    assert all(len(x) == stores_per_tile for x in tile_row_runs)
    sp_share = stores_per_tile // 2
    total_stores = n_tiles * stores_per_tile

    import contextlib
    with contextlib.ExitStack() as ctx:
        Rs = [ctx.enter_context(nc.sbuf_tensor(f"R{k}", [P, n], f32))
              for k in range(NB)]
        Ws = [ctx.enter_context(nc.sbuf_tensor(f"W{k}", [P, span], f32))
              for k in range(NB)]
        # Per-buffer DMA sems: a single shared sem is unsound because the 16
        # per-engine +1s of concurrent DMAs interleave, so a threshold can
        # be crossed before any single DMA fully completed.
        s_lds = [ctx.enter_context(nc.semaphore(f"s_ld{k}"))
                 for k in range(NB)]                      # +16 per load of Rk
        s_sts = [ctx.enter_context(nc.semaphore(f"s_st{k}"))
                 for k in range(NB)]                      # +16 per store of Wk
        s_v = ctx.enter_context(nc.semaphore("s_v"))      # +1 per tile (V copies)
        s_a = ctx.enter_context(nc.semaphore("s_a"))      # +1 per tile (ACT copies)
        s_ms = ctx.enter_context(nc.semaphore("s_ms"))    # +1 per W memset
        block = ctx.enter_context(nc.Block())

        spt = stores_per_tile

        def w_ap(t, d0, ds, cnt, L, rows=P):
            W = Ws[t % NB]
            return bass.AP(W, (d0 - c0), [[span, rows], [ds, cnt], [1, L]])

        def r_ap(t, s0, ss, cnt, L, rows=P):
            R = Rs[t % NB]
            return bass.AP(R, s0, [[n, rows], [ss, cnt], [1, L]])

        @block.sync
        def _(sp):
            for t in range(min(NB, n_tiles)):
                sp.dma_start(Rs[t][:, :], rho[t * P:(t + 1) * P, :]
                             ).then_inc(s_lds[t], 16)
            for t in range(n_tiles):
                k, c = t % NB, t // NB
                sp.wait_ge(s_v, t + 1)
                sp.wait_ge(s_a, t + 1)
                if t < NB:
                    sp.wait_ge(s_ms, t + 1)
                else:
                    sp.wait_ge(s_sts[k], 16 * spt * c)
                W = Ws[k]
                for dr, sr, L in tile_row_runs[t][:sp_share]:
                    sp.dma_start(out[dr:dr + L, c0:c1], W[sr:sr + L, :]
                                 ).then_inc(s_sts[k], 16)
                if t + NB < n_tiles:
                    # R[k] readers (tile t's copies) just finished.
                    sp.wait_ge(s_lds[k], 16 * (c + 1))
                    sp.dma_start(Rs[k][:, :],
                                 rho[(t + NB) * P:(t + NB + 1) * P, :]
                                 ).then_inc(s_lds[k], 16)
            for k in range(NB):
                n_k = len([t for t in range(n_tiles) if t % NB == k])
                sp.wait_ge(s_sts[k], 16 * spt * n_k)

        @block.scalar
        def _(act):
            for t in range(n_tiles):
                k, c = t % NB, t // NB
                act.wait_ge(s_lds[k], 16 * (c + 1))
                if t < NB:
                    act.wait_ge(s_ms, k + 1)
                else:
                    act.wait_ge(s_sts[k], 16 * spt * c)
                    act.wait_ge(s_a, t - NB + 1)
                last = len(pairs_s) - 1
                for i, (d0, s0, ds, ss, cnt, L) in enumerate(pairs_s):
                    ins = act.copy(w_ap(t, d0, ds, cnt, L),
                                   r_ap(t, s0, ss, cnt, L))
                    if i == last:
                        ins.then_inc(s_a, 1)
                act.wait_ge(s_v, t + 1)
                act.wait_ge(s_a, t + 1)
                W = Ws[k]
                for dr, sr, L in tile_row_runs[t][sp_share:]:
                    act.dma_start(out[dr:dr + L, c0:c1], W[sr:sr + L, :]
                                  ).then_inc(s_sts[k], 16)

        @block.vector
        def _(v):
            v.memset(Ws[0][:, :], 0.0).then_inc(s_ms, 1)
            for t in range(n_tiles):
                k, c = t % NB, t // NB
                v.wait_ge(s_lds[k], 16 * (c + 1))
                if t < NB:
                    v.wait_ge(s_ms, k + 1)
                else:
                    v.wait_ge(s_sts[k], 16 * spt * c)
                    v.wait_ge(s_v, t - NB + 1)
                R, W = Rs[k], Ws[k]
                last = len(runs_v) - 1
                for i, (d0, s0, L) in enumerate(runs_v):
                    ins = v.tensor_copy(W[:, d0 - c0:d0 - c0 + L],
                                        R[:, s0:s0 + L])
                    if i == last:
                        ins.then_inc(s_v, 1)
                if t == 0:
                    v.memset(Ws[1][:, :], 0.0).then_inc(s_ms, 1)
                    v.memset(Ws[2][:, :], 0.0).then_inc(s_ms, 1)
                elif t == 1 and NB > 3:
                    v.memset(Ws[3][:, :], 0.0).then_inc(s_ms, 1)

    nc.compile()
    return nc


def kernel(input_state, fock_idx, fock_dim):
    input_state = np.asarray(input_state)
    idx = np.asarray(fock_idx).astype(np.int64)
    D = int(fock_dim)
    B, n, _ = input_state.shape

    nc = _build(idx, D, n)

    out = np.empty((B, D, D), dtype=input_state.dtype)
    for start in range(0, B, N_CORES):
        stop = min(start + N_CORES, B)
        in_maps = [
            {"rho": np.ascontiguousarray(input_state[b], dtype=np.float32)}
            for b in range(start, stop)
        ]
        res = run_bass_kernel_spmd(nc, in_maps,
                                   core_ids=list(range(stop - start)))
        for k, b in enumerate(range(start, stop)):
            out[b] = res.results[k]["out"]
    return out
